# revision 1
# baseline (speedup 1.0000x reference)
"""Trainium2 Bass kernel for nn_CRF_SelfAttention_65627100283470.

Math (validated vs the reference at 1e-6 rel err):
  - The CRF/marginal branch is dead code: softmax over the class dim sums
    to 1, so sum(cluster_features, 0) == sum of context rows.  The output
    is (sum_{f,p} context2) @ cls_W + cls_b.
  - context2 = w2*T2 + w1*(1-w2)*T1 with T_it the per-iteration temporal
    tensors, and w_it per-frame halting weights -> only per-frame sums of
    temporal are needed at the end.
  - QKV projections are shared across overlapping windows; exp(scores)
    blocks are shared across windows (computed per key-frame strip); the
    output projection commutes with overlap-add; softmax denominators come
    from a ones-column appended to V.

Sharding: 8 heads -> 8 cores (perfect balance). One AllReduce of the
partial temporal between the two iterations + one tiny final AllReduce.
"""
import sys
import types

import numpy as np

F, P, H, HEADS, C, NCLS = 18, 128, 256, 8, 32, 625
SCALES = (2, 4, 6)
HD = H // HEADS
NTOK = F * P  # 2304
NCORES = 8

_FP32R = True   # use float32r (full-rate) matmuls for the fp32 operands
_BF16_ATT = True  # exp(scores) strips + V' in bf16 for the A@V matmul


def _enable_ldw_opt():
    """Walrus's LDWEIGHTS dedup is disabled by default in bass_utils;
    enable it (verified numerically by the rel-err gate in test.py)."""
    import concourse.bass_utils as bu

    if getattr(bu, "_ldw_opt_patched", False):
        return
    orig = bu.bir_verify_and_optimise

    def patched(*args, **kwargs):
        import subprocess
        real_run = bu.run_command

        def run_hook(argv, **kw):
            argv = ["--enable-ldw-opt=true" if a == "--enable-ldw-opt=false"
                    else a for a in argv]
            return real_run(argv, **kw)

        bu.run_command = run_hook
        try:
            return orig(*args, **kwargs)
        finally:
            bu.run_command = real_run

    bu.bir_verify_and_optimise = patched
    bu._ldw_opt_patched = True


def _install_ntff_hook():
    """Recreate the missing antenv.axon_hooks so trace=True works."""
    if "antenv.axon_hooks" in sys.modules:
        return
    try:
        import antenv

        mod = types.ModuleType("antenv.axon_hooks")
        mod._hook = None
        mod.set_axon_ntff_profile_hook = lambda h: setattr(mod, "_hook", h)
        mod.get_axon_ntff_profile_hook = lambda: mod._hook
        sys.modules["antenv.axon_hooks"] = mod
        antenv.axon_hooks = mod
        from trn_agent_boot.trn_boot import _ntff_profile_via_ctypes

        mod.set_axon_ntff_profile_hook(
            _ntff_profile_via_ctypes("/opt/axon/libaxon_pjrt.so")
        )
    except Exception:
        pass


def _chunks(n, lim=512, lo=None):
    """Split n into 512-aligned chunks (PSUM bank alignment for matmul)."""
    out = [lim] * (n // lim)
    if n % lim:
        out.append(n % lim)
    return out


def _counts(s):
    nw = F - s + 1
    c = np.zeros(F, np.float32)
    for w in range(nw):
        c[w:w + s] += 1.0
    return c


def build():
    import concourse.bacc as bacc
    import concourse.mybir as mybir
    from concourse.tile import TileContext

    dt = mybir.dt
    f32 = dt.float32
    bf16 = dt.bfloat16
    att_dt = bf16 if _BF16_ATT else f32
    AF = mybir.ActivationFunctionType
    ALU = mybir.AluOpType

    f32r = dt.float32r if _FP32R else f32

    def r(ap):
        return ap

    f32r_d = dt.float32r if _FP32R else f32
    nc = bacc.Bacc("TRN2", target_bir_lowering=False, debug=False,
                   num_devices=NCORES)

    # ---- I/O ----
    xt_in = nc.dram_tensor("xt", [2, 128, NTOK], att_dt, kind="ExternalInput")
    wqkv_in = nc.dram_tensor("wqkv", [3, 2, 128, 97], att_dt, kind="ExternalInput")
    bqkv_in = nc.dram_tensor("bqkv", [3, 97, 1], f32, kind="ExternalInput")
    wo_in = nc.dram_tensor("wo", [3, 32, 256], att_dt, kind="ExternalInput")
    boq_in = nc.dram_tensor("boq", [2, 128, 1], f32, kind="ExternalInput")
    hw_in = nc.dram_tensor("hw", [2, 128, 1], att_dt, kind="ExternalInput")
    nhb_in = nc.dram_tensor("nhb", [1, 1], f32, kind="ExternalInput")
    cinv_in = nc.dram_tensor("cinv", [3, 128, F], f32, kind="ExternalInput")
    clsw_in = nc.dram_tensor("clsw", [2, 128, NCLS], f32, kind="ExternalInput")
    clsb_in = nc.dram_tensor("clsb", [1, NCLS], f32, kind="ExternalInput")
    id_in = nc.dram_tensor("ident", [128, 128], f32r_d, kind="ExternalInput")
    out_d = nc.dram_tensor("out", [1, NCLS], f32, kind="ExternalOutput")

    ar_in = nc.dram_tensor("ar_in", [2, 128, NTOK], att_dt)
    ar_out = nc.dram_tensor("ar_out", [2, 128, NTOK], att_dt, addr_space="Shared")
    ar2_in = nc.dram_tensor("ar2_in", [2, 128, 1], f32)
    ar2_out = nc.dram_tensor("ar2_out", [2, 128, 1], f32, addr_space="Shared")

    inv_sqrt_hd = 1.0 / np.sqrt(np.float32(HD))
    cnts = {s: _counts(s) for s in SCALES}

    with TileContext(nc) as tc:
        with (
            tc.tile_pool(name="pin", bufs=1) as pin,          # persistent SBUF
            tc.tile_pool(name="work", bufs=3) as work,        # rotating SBUF
            tc.tile_pool(name="estr2", bufs=5) as estr2,
            tc.tile_pool(name="estr4", bufs=7) as estr4,
            tc.tile_pool(name="estr6", bufs=9) as estr6,
            tc.tile_pool(name="ppq", bufs=2, space="PSUM") as ppq,
            tc.tile_pool(name="pps", bufs=1, space="PSUM") as pps,
            tc.tile_pool(name="ppa", bufs=1, space="PSUM") as ppa,
            tc.tile_pool(name="ppt", bufs=1, space="PSUM") as ppt,
        ):
            estr = {2: estr2, 4: estr4, 6: estr6}

            # ---- persistent tiles + weight loads ----
            xt = [pin.tile([128, NTOK], att_dt, tag=f"xt{c}", name=f"xt{c}") for c in range(2)]
            wqkv = pin.tile([128, 3 * 2 * 97], att_dt, tag="wqkv")
            bqkv = pin.tile([97, 3], f32, tag="bqkv")
            wo = pin.tile([32, 3 * 256], att_dt, tag="wo")
            boq = pin.tile([128, 2], f32, tag="boq")
            hw_t = pin.tile([128, 2], att_dt, tag="hw")
            nhb = pin.tile([1, 1], f32, tag="nhb")
            cinv = pin.tile([128, 3 * F], f32, tag="cinv")
            clsw = pin.tile([128, 2 * NCLS], f32, tag="clsw")
            clsb = pin.tile([1, NCLS], f32, tag="clsb")
            ident = pin.tile([128, 128], f32r, tag="ident")
            ones_row = pin.tile([1, 128], f32, tag="ones_row")
            ones_col = pin.tile([128, 1], f32, tag="ones_col")
            identb = pin.tile([128, 128], att_dt, tag="identb")

            for c in range(2):
                nc.sync.dma_start(out=xt[c][:], in_=xt_in[c])
            for si in range(3):
                for c in range(2):
                    nc.sync.dma_start(out=wqkv[:, (si * 2 + c) * 97:(si * 2 + c + 1) * 97],
                                      in_=wqkv_in[si, c])
                nc.sync.dma_start(out=bqkv[:, si:si + 1], in_=bqkv_in[si])
                nc.gpsimd.dma_start(out=wo[:, si * 256:(si + 1) * 256], in_=wo_in[si])
                nc.gpsimd.dma_start(out=cinv[:, si * F:(si + 1) * F], in_=cinv_in[si])
            for c in range(2):
                nc.gpsimd.dma_start(out=boq[:, c:c + 1], in_=boq_in[c])
                nc.sync.dma_start(out=hw_t[:, c:c + 1], in_=hw_in[c])
                nc.gpsimd.dma_start(out=clsw[:, c * NCLS:(c + 1) * NCLS], in_=clsw_in[c])
            nc.sync.dma_start(out=nhb[:], in_=nhb_in[:])
            nc.gpsimd.dma_start(out=clsb[:], in_=clsb_in[:])
            nc.gpsimd.dma_start(out=ident[:], in_=id_in[:])
            nc.vector.memset(ones_row[:], 1.0)
            nc.vector.memset(ones_col[:], 1.0)
            nc.vector.tensor_copy(identb[:], ident[:].bitcast(f32))

            qkvT = {s: pin.tile([97, NTOK], att_dt, tag=f"qkvT{s}", name=f"qkvT{s}") for s in SCALES}
            vp = {s: pin.tile([128, F * 33], att_dt, tag=f"vp{s}", name=f"vp{s}") for s in SCALES}
            abar = {s: pin.tile([128, F * 32], f32, tag=f"abar{s}", name=f"abar{s}") for s in SCALES}
            abarT = {s: pin.tile([32, NTOK], att_dt, tag=f"abarT{s}", name=f"abarT{s}") for s in SCALES}
            abarb = {s: pin.tile([128, F * 32], att_dt, tag=f"abarb{s}", name=f"abarb{s}") for s in SCALES}

            # halting state
            ptn = pin.tile([1, F], f32, tag="ptn")
            Rt = pin.tile([1, F], f32, tag="Rt")
            wts = [pin.tile([1, F], f32, tag=f"w{it}", name=f"w{it}") for it in range(2)]
            ssum = [[pin.tile([128, F], f32, tag=f"ssum{it}{c}", name=f"ssum{it}{c}") for c in range(2)]
                    for it in range(2)]
            nc.vector.memset(ptn[:], 0.0)
            nc.vector.memset(Rt[:], 0.0)

            col_cc = _chunks(NTOK)  # [512,512,512,512,256]

            for it in range(2):
                # ---- QKV^T projections for all scales (col 96 of s=2 also
                # carries the halting logits, packed into the same matmul) ----
                for si, s in enumerate(SCALES):
                    off = 0
                    for w_cc in col_cc:
                        pq = ppq.tile([97, 512], f32, tag="pq", name="pq")
                        for kc in range(2):
                            nc.tensor.matmul(
                                pq[:, :w_cc],
                                r(wqkv[:, (si * 2 + kc) * 97:(si * 2 + kc + 1) * 97]),
                                r(xt[kc][:, off:off + w_cc]),
                                start=(kc == 0), stop=(kc == 1))
                        nc.vector.tensor_scalar_add(
                            out=qkvT[s][:, off:off + w_cc], in0=pq[:, :w_cc],
                            scalar1=bqkv[:, si:si + 1])
                        off += w_cc

                # ============ halting probability ============
                elog = work.tile([1, NTOK], f32, tag="elog", bufs=1)
                # exp(-(logit + halt_b)) from the packed row 96 of scale 2
                nc.scalar.activation(elog[:], qkvT[2][96:97, :],
                                     AF.Exp, bias=nhb[:], scale=-1.0)
                nc.vector.tensor_scalar_add(out=elog[:], in0=elog[:], scalar1=1.0)
                ptp = ppt.tile([128, F], f32, tag="pt")
                for f in range(F):
                    nc.tensor.transpose(ptp[:, f:f + 1],
                                        elog[:, f * 128:(f + 1) * 128],
                                        ident[0:1, 0:1].bitcast(f32))
                sig = work.tile([128, F], f32, tag="sig")
                nc.vector.reciprocal(sig[:], ptp[:])
                pp = ppq.tile([1, 512], f32, tag="pq")
                nc.tensor.matmul(pp[:, :F], ones_col[:],
                                 sig[:], start=True, stop=True)
                p_t = work.tile([1, F], f32, tag="p_t")
                nc.vector.tensor_scalar_mul(out=p_t[:], in0=pp[:, :F],
                                            scalar1=1.0 / 128.0)

                # halting state updates (elementwise on [1,F])
                run_in = work.tile([1, F], f32, tag="run_in")
                tmp = work.tile([1, F], f32, tag="tmp")
                tmp2 = work.tile([1, F], f32, tag="tmp2")
                nh = work.tile([1, F], f32, tag="nh")
                run = work.tile([1, F], f32, tag="run")
                nc.vector.tensor_scalar(out=run_in[:], in0=ptn[:], scalar1=1.0,
                                        scalar2=None, op0=ALU.is_lt)
                nc.vector.tensor_tensor(out=tmp[:], in0=p_t[:], in1=run_in[:],
                                        op=ALU.mult)        # p*run_in
                nc.vector.tensor_tensor(out=tmp2[:], in0=ptn[:], in1=tmp[:],
                                        op=ALU.add)         # ptn + p*run_in
                nc.vector.tensor_scalar(out=tmp2[:], in0=tmp2[:], scalar1=0.99,
                                        scalar2=None, op0=ALU.is_gt)  # cond
                nc.vector.tensor_tensor(out=nh[:], in0=tmp2[:], in1=run_in[:],
                                        op=ALU.mult)        # new_halted
                nc.vector.tensor_tensor(out=run[:], in0=run_in[:], in1=nh[:],
                                        op=ALU.subtract)    # run
                nc.vector.tensor_tensor(out=tmp[:], in0=p_t[:], in1=run[:],
                                        op=ALU.mult)        # p*run
                nc.vector.tensor_tensor(out=ptn[:], in0=ptn[:], in1=tmp[:],
                                        op=ALU.add)         # ptn += p*run
                nc.vector.tensor_scalar(out=tmp2[:], in0=ptn[:], scalar1=-1.0,
                                        scalar2=1.0, op0=ALU.mult, op1=ALU.add)
                nc.vector.tensor_tensor(out=tmp2[:], in0=nh[:], in1=tmp2[:],
                                        op=ALU.mult)        # nh*(1-ptn)
                nc.vector.tensor_tensor(out=Rt[:], in0=Rt[:], in1=tmp2[:],
                                        op=ALU.add)         # Rt update
                nc.vector.tensor_tensor(out=tmp2[:], in0=nh[:], in1=Rt[:],
                                        op=ALU.mult)        # nh*Rt
                nc.vector.tensor_tensor(out=ptn[:], in0=ptn[:], in1=tmp2[:],
                                        op=ALU.add)         # ptn += nh*Rt
                nc.vector.tensor_tensor(out=wts[it][:], in0=tmp[:], in1=tmp2[:],
                                        op=ALU.add)         # w = p*run + nh*Rt

                # ============ attention per scale ============
                for si, s in enumerate(SCALES):
                    nw = F - s + 1
                    sP = s * 128
                    kT_t = work.tile([32, NTOK], att_dt, tag="kT", bufs=3, name="kT_t")
                    nc.sync.dma_start(out=kT_t[:], in_=qkvT[s][32:64, :])
                    # V' = [V | ones] in token-major layout
                    nc.vector.memset(vp[s][:], 1.0)
                    for t in range(F):
                        pv = ppt.tile([128, 32], att_dt, tag="pt")
                        nc.tensor.transpose(pv[:],
                                            qkvT[s][64:96, t * 128:(t + 1) * 128],
                                            identb[64:96, 64:96])
                        nc.vector.tensor_copy(vp[s][:, t * 33:t * 33 + 32], pv[:])

                    nc.vector.memset(abar[s][:], 0.0)

                    strips = {}
                    for f2 in range(F):
                        a = max(0, f2 - s + 1)
                        b = min(F - 1, f2 + s - 1)
                        width = b - a + 1
                        ncols = width * 128
                        pstr = pps.tile([128, (2 * s - 1) * 128], f32, tag="pstr", bufs=1)
                        off = 0
                        for w_cc in _chunks(ncols):
                            nc.tensor.matmul(
                                pstr[:, off:off + w_cc],
                                r(kT_t[:, f2 * 128:(f2 + 1) * 128]),
                                r(qkvT[s][0:32, a * 128 + off:a * 128 + off + w_cc]),
                                start=True, stop=True)
                            off += w_cc
                        est = estr[s].tile([128, (2 * s - 1) * 128], att_dt, tag="est")
                        nc.scalar.activation(est[:, :ncols], pstr[:, :ncols],
                                             AF.Exp, scale=inv_sqrt_hd)
                        strips[f2] = (a, est)

                        w = f2 - s + 1
                        if w < 0 or w >= nw:
                            continue
                        # ---- window w: A@V' with denominator column ----
                        pav = ppa.tile([33, 768], f32, tag="pav")
                        av_cc = _chunks(sP)
                        for ji, j in enumerate(range(w, w + s)):
                            aj, ej = strips[j]
                            qoff = (w - aj) * 128
                            off = 0
                            for w_cc in av_cc:
                                nc.tensor.matmul(
                                    pav[:, off:off + w_cc],
                                    vp[s][:, j * 33:(j + 1) * 33],
                                    ej[:, qoff + off:qoff + off + w_cc],
                                    start=(ji == 0), stop=(ji == s - 1))
                                off += w_cc
                        av_sb = work.tile([33, 768], att_dt, tag="av_sb", bufs=3)
                        eng = nc.scalar if (w % 2 == 0) else nc.vector
                        if eng is nc.scalar:
                            nc.scalar.copy(av_sb[:, :sP], pav[:, :sP])
                        else:
                            nc.vector.tensor_copy(av_sb[:, :sP], pav[:, :sP])
                        # transpose to token-major and rescale
                        pt = ppt.tile([128, 6 * 34], att_dt, tag="pt")
                        for qc in range(s):
                            nc.tensor.transpose(
                                pt[:, qc * 34:qc * 34 + 33],
                                av_sb[:, qc * 128:(qc + 1) * 128],
                                identb[0:33, 0:33])
                        ptv = pt[:].rearrange("p (s c) -> p s c", c=34)
                        rcp = work.tile([128, 6], f32, tag="rcp")
                        nc.vector.reciprocal(rcp[:, :s], ptv[:, :s, 32])
                        nc.vector.tensor_tensor(out=rcp[:, :s], in0=rcp[:, :s],
                                                in1=cinv[:, si * F + w:si * F + w + s],
                                                op=ALU.mult)
                        resc = work.tile([128, 6 * 32], f32, tag="resc")
                        rv = resc[:].rearrange("p (s c) -> p s c", c=32)
                        nc.vector.tensor_tensor(
                            out=rv[:, :s, :], in0=ptv[:, :s, 0:32],
                            in1=rcp[:, :s].broadcast_to((128, s, 32)),
                            op=ALU.mult)
                        nc.vector.tensor_tensor(
                            out=abar[s][:, w * 32:(w + s) * 32],
                            in0=abar[s][:, w * 32:(w + s) * 32],
                            in1=resc[:, :s * 32], op=ALU.add)

                    # abar -> abarT (bf16: 1 cyc/row transposes + cheap LDW)
                    nc.vector.tensor_copy(abarb[s][:], abar[s][:])
                    for f in range(F):
                        pat = ppt.tile([32, 128], att_dt, tag="pt")
                        nc.tensor.transpose(pat[:],
                                            abarb[s][:, f * 32:(f + 1) * 32],
                                            identb[:])
                        nc.vector.tensor_copy(abarT[s][:, f * 128:(f + 1) * 128],
                                              pat[:])

                # ============ Wo projection (+0.25 scale, +bias) ============
                for hc in range(2):
                    off = 0
                    for w_cc in col_cc:
                        pw = ppq.tile([128, 512], f32, tag="pq")
                        for si, s in enumerate(SCALES):
                            nc.tensor.matmul(
                                pw[:, :w_cc],
                                r(wo[:, si * 256 + hc * 128:si * 256 + (hc + 1) * 128]),
                                r(abarT[s][:, off:off + w_cc]),
                                start=(si == 0), stop=(si == 2))
                        nc.vector.tensor_scalar(
                            out=xt[hc][:, off:off + w_cc], in0=pw[:, :w_cc],
                            scalar1=0.25, scalar2=boq[:, hc:hc + 1],
                            op0=ALU.mult, op1=ALU.add)
                        if it == 0:
                            nc.sync.dma_start(out=ar_in[hc, :, off:off + w_cc],
                                              in_=xt[hc][:, off:off + w_cc])
                        off += w_cc

                if it == 0:
                    nc.gpsimd.collective_compute(
                        "AllReduce", ALU.add,
                        ins=[ar_in[:]], outs=[ar_out[:]],
                        replica_groups=[list(range(NCORES))])
                    for hc in range(2):
                        nc.sync.dma_start(out=xt[hc][:], in_=ar_out[hc])
                # per-frame sums (it=0: full post-AR temporal; it=1: partial)
                for hc in range(2):
                    nc.vector.tensor_reduce(
                        out=ssum[it][hc][:],
                        in_=xt[hc][:].rearrange("p (f q) -> p f q", q=128),
                        axis=mybir.AxisListType.X, op=ALU.add)

            # ============ final combine ============
            w1, w2 = wts[0], wts[1]
            c1 = work.tile([1, 2 * F], f32, tag="coef")  # [c2 | c1]
            nc.vector.tensor_copy(c1[:, 0:F], w2[:])
            tmpc = work.tile([1, F], f32, tag="tmpc")
            nc.vector.tensor_scalar(out=tmpc[:], in0=w2[:], scalar1=-1.0,
                                    scalar2=1.0, op0=ALU.mult, op1=ALU.add)
            nc.vector.tensor_tensor(out=tmpc[:], in0=tmpc[:], in1=w1[:],
                                    op=ALU.mult)
            nc.vector.tensor_scalar_mul(out=c1[:, F:2 * F], in0=tmpc[:],
                                        scalar1=1.0 / NCORES)
            pc = ppt.tile([128, 2 * F], f32, tag="pt")
            nc.tensor.matmul(pc[:], ones_row[:], c1[:], start=True, stop=True)
            coefb = work.tile([128, 2 * F], f32, tag="coefb")
            nc.vector.tensor_copy(coefb[:], pc[:])
            vpart = [work.tile([128, 1], f32, tag=f"vpart{hc}", name=f"vpart{hc}") for hc in range(2)]
            for hc in range(2):
                t2 = work.tile([128, F], f32, tag="t2")
                nc.vector.tensor_tensor(out=t2[:], in0=ssum[1][hc][:],
                                        in1=coefb[:, 0:F], op=ALU.mult)
                t1 = work.tile([128, F], f32, tag="t1")
                nc.vector.tensor_tensor(out=t1[:], in0=ssum[0][hc][:],
                                        in1=coefb[:, F:2 * F], op=ALU.mult)
                nc.vector.tensor_tensor(out=t2[:], in0=t2[:], in1=t1[:],
                                        op=ALU.add)
                nc.vector.tensor_reduce(out=vpart[hc][:], in_=t2[:],
                                        axis=mybir.AxisListType.X, op=ALU.add)
                nc.sync.dma_start(out=ar2_in[hc], in_=vpart[hc][:])
            nc.gpsimd.collective_compute(
                "AllReduce", ALU.add,
                ins=[ar2_in[:]], outs=[ar2_out[:]],
                replica_groups=[list(range(NCORES))])
            vfull = [work.tile([128, 1], f32, tag=f"vfull{hc}", name=f"vfull{hc}") for hc in range(2)]
            pcls = ppq.tile([1, 512], f32, tag="pq")
            ob = work.tile([1, NCLS], f32, tag="ob")
            for hc in range(2):
                nc.sync.dma_start(out=vfull[hc][:], in_=ar2_out[hc])
            off = 0
            for w_cc in _chunks(NCLS):
                pcls = ppq.tile([1, 512], f32, tag="pq")
                for hc in range(2):
                    nc.tensor.matmul(pcls[:, :w_cc], vfull[hc][:],
                                     clsw[:, hc * NCLS + off:hc * NCLS + off + w_cc],
                                     start=(hc == 0), stop=(hc == 1))
                nc.vector.tensor_tensor(out=ob[:, off:off + w_cc],
                                        in0=pcls[:, :w_cc],
                                        in1=clsb[:, off:off + w_cc], op=ALU.add)
                off += w_cc
            nc.sync.dma_start(out=out_d[:], in_=ob[:])

    nc.compile()
    return nc


_NC_CACHE = None


def _get_nc():
    global _NC_CACHE
    if _NC_CACHE is None:
        _NC_CACHE = build()
    return _NC_CACHE


def _prep_in_maps(inputs):
    emb = np.ascontiguousarray(np.asarray(inputs["multiscale_embed"], np.float32))
    halt_W = np.asarray(inputs["halt_W"], np.float32)
    halt_b = np.asarray(inputs["halt_b"], np.float32)
    cls_W = np.asarray(inputs["cls_W"], np.float32)
    cls_b = np.asarray(inputs["cls_b"], np.float32)
    Wq = np.asarray(inputs["mhsa_Wq"], np.float32)
    bq = np.asarray(inputs["mhsa_bq"], np.float32)
    Wk = np.asarray(inputs["mhsa_Wk"], np.float32)
    bk = np.asarray(inputs["mhsa_bk"], np.float32)
    Wv = np.asarray(inputs["mhsa_Wv"], np.float32)
    bv = np.asarray(inputs["mhsa_bv"], np.float32)
    Wo = np.asarray(inputs["mhsa_Wo"], np.float32)
    bo = np.asarray(inputs["mhsa_bo"], np.float32)

    import ml_dtypes
    bf = ml_dtypes.bfloat16
    xt = np.ascontiguousarray(
        emb.reshape(NTOK, H).T.reshape(2, 128, NTOK)).astype(bf)
    boq = np.ascontiguousarray(
        (0.25 * bo.sum(axis=0)).reshape(2, 128, 1))
    hw = np.ascontiguousarray(halt_W.reshape(2, 128, 1)).astype(bf)
    hwc = halt_W.reshape(2, 128)
    nhb = np.full((1, 1), -float(halt_b[0]), np.float32)
    cinv = np.stack([
        np.repeat((1.0 / _counts(s))[None, :], 128, axis=0) for s in SCALES
    ]).astype(np.float32)
    clsw = np.ascontiguousarray(cls_W.reshape(2, 128, NCLS))
    clsb = cls_b.reshape(1, NCLS).astype(np.float32)
    ident = np.eye(128, dtype=np.float32)

    in_maps = []
    for h in range(NCORES):
        sl = slice(h * HD, (h + 1) * HD)
        wqkv = np.zeros((3, 2, 128, 97), bf)
        bqkv = np.zeros((3, 97, 1), np.float32)
        wo_l = np.zeros((3, 32, 256), bf)
        for si in range(3):
            blk = np.concatenate(
                [Wq[si][:, sl], Wk[si][:, sl], Wv[si][:, sl]], axis=1)  # [256,96]
            wqkv[si, :, :, :96] = blk.reshape(2, 128, 96).astype(bf)
            if si == 0:
                wqkv[si, :, :, 96] = hwc.astype(bf)
            bqkv[si, :96] = np.concatenate(
                [bq[si][sl], bk[si][sl], bv[si][sl]])[:, None]
            wo_l[si] = Wo[si][sl, :].astype(bf)
        in_maps.append({
            "xt": xt, "wqkv": wqkv, "bqkv": bqkv, "wo": wo_l, "boq": boq,
            "hw": hw, "nhb": nhb, "cinv": cinv, "clsw": clsw, "clsb": clsb,
            "ident": ident,
        })
    return in_maps


def run(inputs, trace=False):
    _install_ntff_hook()
    from concourse.bass_utils import run_bass_kernel_spmd

    nc = _get_nc()
    in_maps = _prep_in_maps(inputs)
    res = run_bass_kernel_spmd(nc, in_maps, list(range(NCORES)), trace=trace)
    out = np.asarray(res.results[0]["out"], np.float32)
    return out, res


def kernel(**inputs):
    out, _ = run(inputs, trace=False)
    return out



# revision 2
# speedup vs baseline: 1.0009x; 1.0009x over previous
"""Trainium2 Bass kernel for nn_CRF_SelfAttention_65627100283470.

Math (validated vs the reference at 1e-6 rel err):
  - The CRF/marginal branch is dead code: softmax over the class dim sums
    to 1, so sum(cluster_features, 0) == sum of context rows.  The output
    is (sum_{f,p} context2) @ cls_W + cls_b.
  - context2 = w2*T2 + w1*(1-w2)*T1 with T_it the per-iteration temporal
    tensors, and w_it per-frame halting weights -> only per-frame sums of
    temporal are needed at the end.
  - QKV projections are shared across overlapping windows; exp(scores)
    blocks are shared across windows (computed per key-frame strip); the
    output projection commutes with overlap-add; softmax denominators come
    from a ones-column appended to V.

Sharding: 8 heads -> 8 cores. One AllReduce of the partial temporal
between the two iterations + one tiny final AllReduce.

PE utilisation: score matmuls (K=32) run 4-way row-tiled (kT/q replicated
to partition offsets 0/32/64/96), A@V' (M=33) runs 2-way col-tiled over
window pairs, Wo (K=32) runs 4-way row-tiled against a frame-grouped
transposed layout of abar.
"""
import sys
import types

import numpy as np

F, P, H, HEADS, C, NCLS = 18, 128, 256, 8, 32, 625
SCALES = (2, 4, 6)
HD = H // HEADS
NTOK = F * P  # 2304
NCORES = 8
NG = (F + 3) // 4  # 4-frame groups: 5


def _install_ntff_hook():
    """Recreate the missing antenv.axon_hooks so trace=True works."""
    if "antenv.axon_hooks" in sys.modules:
        return
    try:
        import antenv

        mod = types.ModuleType("antenv.axon_hooks")
        mod._hook = None
        mod.set_axon_ntff_profile_hook = lambda h: setattr(mod, "_hook", h)
        mod.get_axon_ntff_profile_hook = lambda: mod._hook
        sys.modules["antenv.axon_hooks"] = mod
        antenv.axon_hooks = mod
        from trn_agent_boot.trn_boot import _ntff_profile_via_ctypes

        mod.set_axon_ntff_profile_hook(
            _ntff_profile_via_ctypes("/opt/axon/libaxon_pjrt.so")
        )
    except Exception:
        pass


def _chunks(n, lim=512):
    out = [lim] * (n // lim)
    if n % lim:
        out.append(n % lim)
    return out


def _counts(s):
    nw = F - s + 1
    c = np.zeros(F, np.float32)
    for w in range(nw):
        c[w:w + s] += 1.0
    return c


def build():
    import concourse.bacc as bacc
    import concourse.mybir as mybir
    from concourse.tile import TileContext

    dt = mybir.dt
    f32 = dt.float32
    bf16 = dt.bfloat16
    AF = mybir.ActivationFunctionType
    ALU = mybir.AluOpType
    f32r = dt.float32r

    nc = bacc.Bacc("TRN2", target_bir_lowering=False, debug=False,
                   num_devices=NCORES)

    # ---- I/O ----
    xt_in = nc.dram_tensor("xt", [2, 128, NTOK], bf16, kind="ExternalInput")
    wqkv_in = nc.dram_tensor("wqkv", [3, 2, 128, 97], bf16, kind="ExternalInput")
    bqkv_in = nc.dram_tensor("bqkv", [3, 97, 1], f32, kind="ExternalInput")
    wo4_in = nc.dram_tensor("wo4", [128, 768], bf16, kind="ExternalInput")
    boq_in = nc.dram_tensor("boq", [2, 128, 1], f32, kind="ExternalInput")
    nhb_in = nc.dram_tensor("nhb", [1, 1], f32, kind="ExternalInput")
    cinvT2_in = nc.dram_tensor("cinvT2", [3, 128, NG * 128], f32,
                               kind="ExternalInput")
    clsw_in = nc.dram_tensor("clsw", [2, 128, NCLS], f32, kind="ExternalInput")
    clsb_in = nc.dram_tensor("clsb", [1, NCLS], f32, kind="ExternalInput")
    id_in = nc.dram_tensor("ident", [128, 128], f32r, kind="ExternalInput")
    out_d = nc.dram_tensor("out", [1, NCLS], f32, kind="ExternalOutput")

    ar_in = nc.dram_tensor("ar_in", [2, 128, NTOK], bf16)
    ar_out = nc.dram_tensor("ar_out", [2, 128, NTOK], bf16, addr_space="Shared")
    ar2_in = nc.dram_tensor("ar2_in", [2, 128, 1], f32)
    ar2_out = nc.dram_tensor("ar2_out", [2, 128, 1], f32, addr_space="Shared")

    inv_sqrt_hd = 1.0 / np.sqrt(np.float32(HD))

    with TileContext(nc) as tc:
        with (
            tc.tile_pool(name="pin", bufs=1) as pin,          # persistent SBUF
            tc.tile_pool(name="work", bufs=3) as work,        # rotating SBUF
            tc.tile_pool(name="estr2", bufs=5) as estr2,
            tc.tile_pool(name="estr4", bufs=7) as estr4,
            tc.tile_pool(name="estr6", bufs=9) as estr6,
            tc.tile_pool(name="ppq", bufs=2, space="PSUM") as ppq,    # 2 banks
            tc.tile_pool(name="pps", bufs=1, space="PSUM") as pps,    # 3 banks
            tc.tile_pool(name="pav", bufs=1, space="PSUM") as pavp,   # 2 banks
            tc.tile_pool(name="ppt", bufs=1, space="PSUM") as ppt,    # 1 bank
        ):
            estr = {2: estr2, 4: estr4, 6: estr6}

            # ---- persistent tiles + weight loads ----
            xt = [pin.tile([128, NTOK], bf16, tag=f"xt{c}", name=f"xt{c}")
                  for c in range(2)]
            wqkv = pin.tile([128, 3 * 2 * 97], bf16, tag="wqkv")
            bqkv = pin.tile([97, 3], f32, tag="bqkv")
            wo4 = pin.tile([128, 768], bf16, tag="wo4")
            boq = pin.tile([128, 2], f32, tag="boq")
            nhb = pin.tile([1, 1], f32, tag="nhb")
            cinvT2 = pin.tile([128, 3 * NG * 128], f32, tag="cinvT2")
            clsw = pin.tile([128, 2 * NCLS], f32, tag="clsw")
            clsb = pin.tile([1, NCLS], f32, tag="clsb")
            ident = pin.tile([128, 128], f32r, tag="ident")
            ones_row = pin.tile([1, 128], f32, tag="ones_row")
            ones_col = pin.tile([128, 1], f32, tag="ones_col")
            identb = pin.tile([128, 128], bf16, tag="identb")

            for c in range(2):
                nc.sync.dma_start(out=xt[c][:], in_=xt_in[c])
            for si in range(3):
                for c in range(2):
                    nc.sync.dma_start(
                        out=wqkv[:, (si * 2 + c) * 97:(si * 2 + c + 1) * 97],
                        in_=wqkv_in[si, c])
                nc.sync.dma_start(out=bqkv[:, si:si + 1], in_=bqkv_in[si])
                nc.gpsimd.dma_start(
                    out=cinvT2[:, si * NG * 128:(si + 1) * NG * 128],
                    in_=cinvT2_in[si])
            nc.gpsimd.dma_start(out=wo4[:], in_=wo4_in[:])
            for c in range(2):
                nc.gpsimd.dma_start(out=boq[:, c:c + 1], in_=boq_in[c])
                nc.gpsimd.dma_start(out=clsw[:, c * NCLS:(c + 1) * NCLS],
                                    in_=clsw_in[c])
            nc.sync.dma_start(out=nhb[:], in_=nhb_in[:])
            nc.gpsimd.dma_start(out=clsb[:], in_=clsb_in[:])
            nc.gpsimd.dma_start(out=ident[:], in_=id_in[:])
            nc.vector.memset(ones_row[:], 1.0)
            nc.vector.memset(ones_col[:], 1.0)
            nc.vector.tensor_copy(identb[:], ident[:].bitcast(f32))

            qkvT = {s: pin.tile([97, NTOK], bf16, tag=f"qkvT{s}", name=f"qkvT{s}")
                    for s in SCALES}
            q4 = {s: pin.tile([128, NTOK], bf16, tag=f"q4{s}", name=f"q4{s}")
                  for s in SCALES}
            kT4 = {s: pin.tile([128, NTOK], bf16, tag=f"kT4{s}", name=f"kT4{s}")
                   for s in SCALES}
            v4 = {s: pin.tile([128, NTOK], bf16, tag=f"v4{s}", name=f"v4{s}")
                  for s in SCALES}
            vp = {s: pin.tile([128, F * 33], bf16, tag=f"vp{s}", name=f"vp{s}")
                  for s in SCALES}
            abar = {s: pin.tile([128, F * 32], f32, tag=f"abar{s}",
                                name=f"abar{s}") for s in SCALES}
            abT = {s: pin.tile([128, NG * 128], bf16, tag=f"abT{s}",
                               name=f"abT{s}") for s in SCALES}

            # halting state
            ptn = pin.tile([1, F], f32, tag="ptn")
            Rt = pin.tile([1, F], f32, tag="Rt")
            wts = [pin.tile([1, F], f32, tag=f"w{it}", name=f"w{it}")
                   for it in range(2)]
            ssum = [[pin.tile([128, F], f32, tag=f"ssum{it}{c}",
                              name=f"ssum{it}{c}") for c in range(2)]
                    for it in range(2)]
            nc.vector.memset(ptn[:], 0.0)
            nc.vector.memset(Rt[:], 0.0)
            for s in SCALES:
                nc.vector.memset(vp[s][:], 1.0)

            col_cc = _chunks(NTOK)  # [512]*4 + [256]

            def scatter4(dst, src_rows, queue):
                """DMA src [32, NTOK] into dst[32r] for frame f = r mod 4."""
                sv = src_rows[:, 0:16 * 128].rearrange(
                    "p (g r q) -> p g r q", r=4, q=128)
                for r in range(4):
                    dv = dst[32 * r:32 * r + 32, 0:16 * 128].rearrange(
                        "p (g r q) -> p g r q", r=4, q=128)
                    queue.dma_start(out=dv[:, :, r, :], in_=sv[:, :, r, :])
                for f in (16, 17):
                    r = f % 4
                    queue.dma_start(
                        out=dst[32 * r:32 * r + 32, f * 128:(f + 1) * 128],
                        in_=src_rows[:, f * 128:(f + 1) * 128])

            for it in range(2):
                # ============ QKV^T projections (all scales) ============
                for si, s in enumerate(SCALES):
                    off = 0
                    for w_cc in col_cc:
                        pq = ppq.tile([128, 512], f32, tag="pq", name="pq")
                        for kc in range(2):
                            nc.tensor.matmul(
                                pq[0:97, :w_cc],
                                wqkv[:, (si * 2 + kc) * 97:(si * 2 + kc + 1) * 97],
                                xt[kc][:, off:off + w_cc],
                                start=(kc == 0), stop=(kc == 1))
                        nc.vector.tensor_scalar_add(
                            out=qkvT[s][:, off:off + w_cc], in0=pq[0:97, :w_cc],
                            scalar1=bqkv[:, si:si + 1])
                        off += w_cc
                    # fan out Q to partition offsets 32/64/96, scatter K/V
                    for r in range(1, 4):
                        nc.sync.dma_start(out=q4[s][32 * r:32 * r + 32, :],
                                          in_=qkvT[s][0:32, :])
                    scatter4(kT4[s], qkvT[s][32:64, :], nc.gpsimd)
                    scatter4(v4[s], qkvT[s][64:96, :], nc.sync)

                # ============ halting probability ============
                elog = work.tile([1, NTOK], f32, tag="elog", bufs=1)
                nc.scalar.activation(elog[:], qkvT[2][96:97, :],
                                     AF.Exp, bias=nhb[:], scale=-1.0)
                nc.vector.tensor_scalar_add(out=elog[:], in0=elog[:],
                                            scalar1=1.0)
                ptp = ppt.tile([128, F], f32, tag="ptt")
                for f in range(F):
                    nc.tensor.transpose(ptp[:, f:f + 1],
                                        elog[:, f * 128:(f + 1) * 128],
                                        ident[0:1, 0:1].bitcast(f32))
                sig = work.tile([128, F], f32, tag="sig")
                nc.vector.reciprocal(sig[:], ptp[:])
                pp = ppq.tile([128, 512], f32, tag="pq", name="pq")
                nc.tensor.matmul(pp[0:1, :F], ones_col[:],
                                 sig[:], start=True, stop=True)
                p_t = work.tile([1, F], f32, tag="p_t")
                nc.vector.tensor_scalar_mul(out=p_t[:], in0=pp[0:1, :F],
                                            scalar1=1.0 / 128.0)

                run_in = work.tile([1, F], f32, tag="run_in")
                tmp = work.tile([1, F], f32, tag="tmp")
                tmp2 = work.tile([1, F], f32, tag="tmp2")
                nh = work.tile([1, F], f32, tag="nh")
                run = work.tile([1, F], f32, tag="run")
                nc.vector.tensor_scalar(out=run_in[:], in0=ptn[:], scalar1=1.0,
                                        scalar2=None, op0=ALU.is_lt)
                nc.vector.tensor_tensor(out=tmp[:], in0=p_t[:], in1=run_in[:],
                                        op=ALU.mult)
                nc.vector.tensor_tensor(out=tmp2[:], in0=ptn[:], in1=tmp[:],
                                        op=ALU.add)
                nc.vector.tensor_scalar(out=tmp2[:], in0=tmp2[:], scalar1=0.99,
                                        scalar2=None, op0=ALU.is_gt)
                nc.vector.tensor_tensor(out=nh[:], in0=tmp2[:], in1=run_in[:],
                                        op=ALU.mult)
                nc.vector.tensor_tensor(out=run[:], in0=run_in[:], in1=nh[:],
                                        op=ALU.subtract)
                nc.vector.tensor_tensor(out=tmp[:], in0=p_t[:], in1=run[:],
                                        op=ALU.mult)
                nc.vector.tensor_tensor(out=ptn[:], in0=ptn[:], in1=tmp[:],
                                        op=ALU.add)
                nc.vector.tensor_scalar(out=tmp2[:], in0=ptn[:], scalar1=-1.0,
                                        scalar2=1.0, op0=ALU.mult, op1=ALU.add)
                nc.vector.tensor_tensor(out=tmp2[:], in0=nh[:], in1=tmp2[:],
                                        op=ALU.mult)
                nc.vector.tensor_tensor(out=Rt[:], in0=Rt[:], in1=tmp2[:],
                                        op=ALU.add)
                nc.vector.tensor_tensor(out=tmp2[:], in0=nh[:], in1=Rt[:],
                                        op=ALU.mult)
                nc.vector.tensor_tensor(out=ptn[:], in0=ptn[:], in1=tmp2[:],
                                        op=ALU.add)
                nc.vector.tensor_tensor(out=wts[it][:], in0=tmp[:], in1=tmp2[:],
                                        op=ALU.add)

                # ============ attention per scale ============
                for si, s in enumerate(SCALES):
                    nw = F - s + 1
                    sP = s * 128
                    nc.vector.memset(abar[s][:], 0.0)

                    # window pairs (wA, wB) triggered after strip wB+s-1
                    pair_at = {}
                    for pi in range(0, nw, 2):
                        wA, wB = pi, (pi + 1 if pi + 1 < nw else None)
                        trig = (wB if wB is not None else wA) + s - 1
                        pair_at[trig] = (wA, wB)

                    strips = {}
                    for f2 in range(F):
                        # V' transposes for frames (f2, f2+1), 2 per step
                        if f2 % 2 == 0:
                            vfr = [f for f in (f2, f2 + 1) if f < F]
                            pv = ppt.tile([128, 68], bf16, tag="ptt",
                                          name="ptt_v")
                            for fi, f in enumerate(vfr):
                                r = f % 4
                                nc.tensor.transpose(
                                    pv[:, fi * 33:fi * 33 + 32],
                                    v4[s][32 * r:32 * r + 32,
                                          f * 128:(f + 1) * 128],
                                    identb[32 * r:32 * r + 32,
                                           32 * r:32 * r + 32],
                                    tile_position=(32 * r, 0))
                            pvv = pv[:, 0:len(vfr) * 33].rearrange(
                                "p (f c) -> p f c", c=33)
                            vpv = vp[s][:, vfr[0] * 33:
                                        (vfr[0] + len(vfr)) * 33].rearrange(
                                "p (f c) -> p f c", c=33)
                            nc.vector.tensor_copy(vpv[:, :, 0:32],
                                                  pvv[:, :, 0:32])

                        # ---- scores strip f2 (4-way row-tiled) ----
                        a = max(0, f2 - s + 1)
                        b = min(F - 1, f2 + s - 1)
                        ncols = (b - a + 1) * 128
                        r = f2 % 4
                        lhs = kT4[s][32 * r:32 * r + 32,
                                     f2 * 128:(f2 + 1) * 128]
                        mov = (qkvT[s] if r == 0 else q4[s])
                        pstr = pps.tile([128, 11 * 128], f32, tag="pstr",
                                        name="pstr")
                        off = 0
                        for w_cc in _chunks(ncols):
                            nc.tensor.matmul(
                                pstr[:, off:off + w_cc], lhs,
                                mov[32 * r:32 * r + 32,
                                    a * 128 + off:a * 128 + off + w_cc],
                                start=True, stop=True,
                                tile_position=(32 * r, 0))
                            off += w_cc
                        est = estr[s].tile([128, (2 * s - 1) * 128], bf16,
                                           tag="est")
                        nc.scalar.activation(est[:, :ncols], pstr[:, :ncols],
                                             AF.Exp, scale=inv_sqrt_hd)
                        strips[f2] = (a, est)

                        if f2 not in pair_at:
                            continue
                        wA, wB = pair_at[f2]
                        wins = [(0, wA)] + ([(1, wB)] if wB is not None else [])
                        # ---- A@V' for the window pair (2-way col-tiled) ----
                        pav = pavp.tile([128, 768], f32, tag="pav", name="pav")
                        av_cc = _chunks(sP)
                        for ji in range(s):
                            off = 0
                            for w_cc in av_cc:
                                for w2, w in wins:
                                    j = w + ji
                                    aj, ej = strips[j]
                                    qoff = (w - aj) * 128
                                    nc.tensor.matmul(
                                        pav[64 * w2:64 * w2 + 33,
                                            off:off + w_cc],
                                        vp[s][:, j * 33:(j + 1) * 33],
                                        ej[:, qoff + off:qoff + off + w_cc],
                                        start=(ji == 0), stop=(ji == s - 1),
                                        tile_position=(0, 64 * w2))
                                off += w_cc
                        nrow = 97 if wB is not None else 33
                        av_sb = work.tile([128, 768], bf16, tag="av_sb",
                                          bufs=3)
                        if (wA // 2) % 2 == 0:
                            nc.scalar.copy(av_sb[0:nrow, :sP],
                                           pav[0:nrow, :sP])
                        else:
                            nc.vector.tensor_copy(av_sb[0:nrow, :sP],
                                                  pav[0:nrow, :sP])
                        # ---- transpose to token-major (2-way row-tiled) ----
                        ptw = ppt.tile([128, 408], bf16, tag="ptt",
                                       name="ptt_w")
                        for w2, w in wins:
                            base = 64 * w2
                            for qc in range(s):
                                nc.tensor.transpose(
                                    ptw[:, (w2 * s + qc) * 34:
                                        (w2 * s + qc) * 34 + 33],
                                    av_sb[base:base + 33,
                                          qc * 128:(qc + 1) * 128],
                                    identb[base:base + 33, base:base + 33],
                                    tile_position=(base, 0))
                        nwin = len(wins)
                        ptv = ptw[:, 0:nwin * s * 34].rearrange(
                            "p (w c) -> p w c", c=34)
                        rcp = work.tile([128, 12], f32, tag="rcp")
                        nc.vector.reciprocal(rcp[:, :nwin * s],
                                             ptv[:, :, 32])
                        resc = work.tile([128, 12 * 32], f32, tag="resc")
                        rv = resc[:, 0:nwin * s * 32].rearrange(
                            "p (w c) -> p w c", c=32)
                        nc.vector.tensor_tensor(
                            out=rv[:], in0=ptv[:, :, 0:32],
                            in1=rcp[:, :nwin * s].broadcast_to(
                                (128, nwin * s, 32)),
                            op=ALU.mult)
                        for w2, w in wins:
                            nc.vector.tensor_tensor(
                                out=abar[s][:, w * 32:(w + s) * 32],
                                in0=abar[s][:, w * 32:(w + s) * 32],
                                in1=resc[:, w2 * s * 32:(w2 + 1) * s * 32],
                                op=ALU.add)

                    # ---- abar -> frame-grouped transpose (4 frames/128) ----
                    for g in range(NG):
                        gw = min(4, F - 4 * g) * 32
                        pg = ppt.tile([128, 128], f32, tag="ptt", name="ptt_g")
                        nc.tensor.transpose(
                            pg[0:gw, :],
                            abar[s][:, g * 128:g * 128 + gw],
                            ident[:].bitcast(f32))
                        nc.vector.tensor_tensor(
                            out=abT[s][0:gw, g * 128:(g + 1) * 128],
                            in0=pg[0:gw, :],
                            in1=cinvT2[0:gw, (si * NG + g) * 128:
                                       (si * NG + g + 1) * 128],
                            op=ALU.mult)

                # ============ Wo projection (4-way row-tiled) ============
                for hc in range(2):
                    for g in range(NG):
                        nfr = min(4, F - 4 * g)
                        w_cc = nfr * 128
                        pw = ppq.tile([128, 512], f32, tag="pq", name="pq")
                        for si, s in enumerate(SCALES):
                            for j in range(nfr):
                                nc.tensor.matmul(
                                    pw[:, j * 128:(j + 1) * 128],
                                    wo4[32 * j:32 * j + 32,
                                        (si * 2 + hc) * 128:
                                        (si * 2 + hc + 1) * 128],
                                    abT[s][32 * j:32 * j + 32,
                                           g * 128:(g + 1) * 128],
                                    start=(si == 0), stop=(si == 2),
                                    tile_position=(32 * j, 0))
                        off = g * 512
                        nc.vector.tensor_scalar(
                            out=xt[hc][:, off:off + w_cc], in0=pw[:, :w_cc],
                            scalar1=0.25, scalar2=boq[:, hc:hc + 1],
                            op0=ALU.mult, op1=ALU.add)
                        if it == 0:
                            nc.sync.dma_start(out=ar_in[hc, :, off:off + w_cc],
                                              in_=xt[hc][:, off:off + w_cc])

                if it == 0:
                    nc.gpsimd.collective_compute(
                        "AllReduce", ALU.add,
                        ins=[ar_in[:]], outs=[ar_out[:]],
                        replica_groups=[list(range(NCORES))])
                    for hc in range(2):
                        nc.sync.dma_start(out=xt[hc][:], in_=ar_out[hc])
                for hc in range(2):
                    nc.vector.tensor_reduce(
                        out=ssum[it][hc][:],
                        in_=xt[hc][:].rearrange("p (f q) -> p f q", q=128),
                        axis=mybir.AxisListType.X, op=ALU.add)

            # ============ final combine ============
            w1, w2 = wts[0], wts[1]
            c1 = work.tile([1, 2 * F], f32, tag="coef")
            nc.vector.tensor_copy(c1[:, 0:F], w2[:])
            tmpc = work.tile([1, F], f32, tag="tmpc")
            nc.vector.tensor_scalar(out=tmpc[:], in0=w2[:], scalar1=-1.0,
                                    scalar2=1.0, op0=ALU.mult, op1=ALU.add)
            nc.vector.tensor_tensor(out=tmpc[:], in0=tmpc[:], in1=w1[:],
                                    op=ALU.mult)
            nc.vector.tensor_scalar_mul(out=c1[:, F:2 * F], in0=tmpc[:],
                                        scalar1=1.0 / NCORES)
            pc = ppt.tile([128, 2 * F], f32, tag="ptt", name="ptt_c")
            nc.tensor.matmul(pc[:], ones_row[:], c1[:], start=True, stop=True)
            coefb = work.tile([128, 2 * F], f32, tag="coefb")
            nc.vector.tensor_copy(coefb[:], pc[:])
            vpart = [work.tile([128, 1], f32, tag=f"vpart{hc}",
                               name=f"vpart{hc}") for hc in range(2)]
            for hc in range(2):
                t2 = work.tile([128, F], f32, tag="t2")
                nc.vector.tensor_tensor(out=t2[:], in0=ssum[1][hc][:],
                                        in1=coefb[:, 0:F], op=ALU.mult)
                t1 = work.tile([128, F], f32, tag="t1")
                nc.vector.tensor_tensor(out=t1[:], in0=ssum[0][hc][:],
                                        in1=coefb[:, F:2 * F], op=ALU.mult)
                nc.vector.tensor_tensor(out=t2[:], in0=t2[:], in1=t1[:],
                                        op=ALU.add)
                nc.vector.tensor_reduce(out=vpart[hc][:], in_=t2[:],
                                        axis=mybir.AxisListType.X, op=ALU.add)
                nc.sync.dma_start(out=ar2_in[hc], in_=vpart[hc][:])
            nc.gpsimd.collective_compute(
                "AllReduce", ALU.add,
                ins=[ar2_in[:]], outs=[ar2_out[:]],
                replica_groups=[list(range(NCORES))])
            vfull = [work.tile([128, 1], f32, tag=f"vfull{hc}",
                               name=f"vfull{hc}") for hc in range(2)]
            ob = work.tile([1, NCLS], f32, tag="ob")
            for hc in range(2):
                nc.sync.dma_start(out=vfull[hc][:], in_=ar2_out[hc])
            off = 0
            for w_cc in _chunks(NCLS):
                pcls = ppq.tile([128, 512], f32, tag="pq", name="pq")
                for hc in range(2):
                    nc.tensor.matmul(pcls[0:1, :w_cc], vfull[hc][:],
                                     clsw[:, hc * NCLS + off:
                                          hc * NCLS + off + w_cc],
                                     start=(hc == 0), stop=(hc == 1))
                nc.vector.tensor_tensor(out=ob[:, off:off + w_cc],
                                        in0=pcls[0:1, :w_cc],
                                        in1=clsb[:, off:off + w_cc],
                                        op=ALU.add)
                off += w_cc
            nc.sync.dma_start(out=out_d[:], in_=ob[:])

    nc.compile()
    return nc


_NC_CACHE = None


def _get_nc():
    global _NC_CACHE
    if _NC_CACHE is None:
        _NC_CACHE = build()
    return _NC_CACHE


def _prep_in_maps(inputs):
    emb = np.ascontiguousarray(np.asarray(inputs["multiscale_embed"], np.float32))
    halt_W = np.asarray(inputs["halt_W"], np.float32)
    halt_b = np.asarray(inputs["halt_b"], np.float32)
    cls_W = np.asarray(inputs["cls_W"], np.float32)
    cls_b = np.asarray(inputs["cls_b"], np.float32)
    Wq = np.asarray(inputs["mhsa_Wq"], np.float32)
    bq = np.asarray(inputs["mhsa_bq"], np.float32)
    Wk = np.asarray(inputs["mhsa_Wk"], np.float32)
    bk = np.asarray(inputs["mhsa_bk"], np.float32)
    Wv = np.asarray(inputs["mhsa_Wv"], np.float32)
    bv = np.asarray(inputs["mhsa_bv"], np.float32)
    Wo = np.asarray(inputs["mhsa_Wo"], np.float32)
    bo = np.asarray(inputs["mhsa_bo"], np.float32)

    import ml_dtypes
    bf = ml_dtypes.bfloat16
    xt = np.ascontiguousarray(
        emb.reshape(NTOK, H).T.reshape(2, 128, NTOK)).astype(bf)
    boq = np.ascontiguousarray(
        (0.25 * bo.sum(axis=0)).reshape(2, 128, 1))
    hwc = halt_W.reshape(2, 128)
    nhb = np.full((1, 1), -float(halt_b[0]), np.float32)
    # cinvT2[si]: partition (j*32+d), col g*128+t -> 1/counts_s[4g+j]
    cinvT2 = np.zeros((3, 128, NG * 128), np.float32)
    for si, s in enumerate(SCALES):
        cs = 1.0 / _counts(s)
        for g in range(NG):
            for j in range(min(4, F - 4 * g)):
                cinvT2[si, j * 32:(j + 1) * 32,
                       g * 128:(g + 1) * 128] = cs[4 * g + j]
    clsw = np.ascontiguousarray(cls_W.reshape(2, 128, NCLS))
    clsb = cls_b.reshape(1, NCLS).astype(np.float32)
    ident = np.eye(128, dtype=np.float32)

    in_maps = []
    for h in range(NCORES):
        sl = slice(h * HD, (h + 1) * HD)
        wqkv = np.zeros((3, 2, 128, 97), bf)
        bqkv = np.zeros((3, 97, 1), np.float32)
        wo4 = np.zeros((128, 768), bf)
        for si in range(3):
            blk = np.concatenate(
                [Wq[si][:, sl], Wk[si][:, sl], Wv[si][:, sl]], axis=1)
            wqkv[si, :, :, :96] = blk.reshape(2, 128, 96).astype(bf)
            if si == 0:
                wqkv[si, :, :, 96] = hwc.astype(bf)
            bqkv[si, :96] = np.concatenate(
                [bq[si][sl], bk[si][sl], bv[si][sl]])[:, None]
            for hc in range(2):
                for j in range(4):
                    wo4[32 * j:32 * j + 32,
                        (si * 2 + hc) * 128:(si * 2 + hc + 1) * 128] = \
                        Wo[si][sl, hc * 128:(hc + 1) * 128].astype(bf)
        in_maps.append({
            "xt": xt, "wqkv": wqkv, "bqkv": bqkv, "wo4": wo4, "boq": boq,
            "nhb": nhb, "cinvT2": cinvT2, "clsw": clsw, "clsb": clsb,
            "ident": ident,
        })
    return in_maps


def run(inputs, trace=False):
    _install_ntff_hook()
    from concourse.bass_utils import run_bass_kernel_spmd

    nc = _get_nc()
    in_maps = _prep_in_maps(inputs)
    res = run_bass_kernel_spmd(nc, in_maps, list(range(NCORES)), trace=trace)
    out = np.asarray(res.results[0]["out"], np.float32)
    return out, res


def kernel(**inputs):
    out, _ = run(inputs, trace=False)
    return out


# revision 21
# speedup vs baseline: 1.0897x; 1.0887x over previous
"""Trainium2 Bass kernel for nn_CRF_SelfAttention_65627100283470.

Math (validated vs the reference at 1e-6 rel err):
  - The CRF/marginal branch is dead code: softmax over the class dim sums
    to 1, so sum(cluster_features, 0) == sum of context rows.  The output
    is (sum_{f,p} context2) @ cls_W + cls_b.
  - context2 = w2*T2 + w1*(1-w2)*T1 with T_it the per-iteration temporal
    tensors, and w_it per-frame halting weights -> only per-frame sums of
    temporal are needed at the end.
  - QKV projections are shared across overlapping windows; exp(scores)
    blocks are shared across windows (computed per key-frame strip); the
    output projection commutes with overlap-add; softmax denominators come
    from a ones-column appended to V.

Sharding: 8 heads -> 8 cores. One AllReduce of the partial temporal
between the two iterations + one tiny final AllReduce.

PE utilisation: score matmuls (K=32) run 4-way row-tiled (kT/q replicated
to partition offsets 0/32/64/96), A@V' (M=33) runs 2-way col-tiled over
window pairs, Wo (K=32) runs 4-way row-tiled against a frame-grouped
transposed layout of abar.
"""
import sys
import types

import numpy as np

F, P, H, HEADS, C, NCLS = 18, 128, 256, 8, 32, 625
SCALES = (2, 4, 6)
HD = H // HEADS
NTOK = F * P  # 2304
NCORES = 8
NG = (F + 3) // 4  # 4-frame groups: 5

import os
DBG = os.environ.get("K_DBG", "0") == "1"
R4 = os.environ.get("K_R4", "1") == "1"   # 4-way row-tiled scores
C2 = int(os.environ.get("K_C2", "1"))     # 2-way col-tiled A@V pairs (2=solo-alt)
V4 = os.environ.get("K_V4", "1") == "1"   # 4-way tiled V' transposes
W4 = os.environ.get("K_W4", "1") == "1"   # 4-way row-tiled Wo


def _install_ntff_hook():
    """Recreate the missing antenv.axon_hooks so trace=True works."""
    if "antenv.axon_hooks" in sys.modules:
        return
    try:
        import antenv

        mod = types.ModuleType("antenv.axon_hooks")
        mod._hook = None
        mod.set_axon_ntff_profile_hook = lambda h: setattr(mod, "_hook", h)
        mod.get_axon_ntff_profile_hook = lambda: mod._hook
        sys.modules["antenv.axon_hooks"] = mod
        antenv.axon_hooks = mod
        from trn_agent_boot.trn_boot import _ntff_profile_via_ctypes

        mod.set_axon_ntff_profile_hook(
            _ntff_profile_via_ctypes("/opt/axon/libaxon_pjrt.so")
        )
    except Exception:
        pass


def _chunks(n, lim=512):
    out = [lim] * (n // lim)
    if n % lim:
        out.append(n % lim)
    return out


def _counts(s):
    nw = F - s + 1
    c = np.zeros(F, np.float32)
    for w in range(nw):
        c[w:w + s] += 1.0
    return c


def build():
    import concourse.bacc as bacc
    import concourse.mybir as mybir
    from concourse.tile import TileContext

    dt = mybir.dt
    f32 = dt.float32
    bf16 = dt.bfloat16
    AF = mybir.ActivationFunctionType
    ALU = mybir.AluOpType
    f32r = dt.float32r

    nc = bacc.Bacc("TRN2", target_bir_lowering=False, debug=False,
                   num_devices=NCORES)

    # ---- I/O ----
    xt_in = nc.dram_tensor("xt", [2, 128, NTOK], bf16, kind="ExternalInput")
    wqkv_in = nc.dram_tensor("wqkv", [3, 2, 128, 97], bf16, kind="ExternalInput")
    bqkv_in = nc.dram_tensor("bqkv", [3, 97, 1], f32, kind="ExternalInput")
    wo3_in = nc.dram_tensor("wo3", [128, 256], bf16, kind="ExternalInput")
    boq_in = nc.dram_tensor("boq", [2, 128, 1], f32, kind="ExternalInput")
    nhb_in = nc.dram_tensor("nhb", [1, 1], f32, kind="ExternalInput")
    cinvT2_in = nc.dram_tensor("cinvT2", [3, 128, NG * 128], f32,
                               kind="ExternalInput")
    clsw_in = nc.dram_tensor("clsw", [2, 128, NCLS], f32, kind="ExternalInput")
    clsb_in = nc.dram_tensor("clsb", [1, NCLS], f32, kind="ExternalInput")
    id_in = nc.dram_tensor("ident", [128, 128], f32r, kind="ExternalInput")
    out_d = nc.dram_tensor("out", [1, NCLS], f32, kind="ExternalOutput")
    if DBG:
        dbg_qkvT = nc.dram_tensor("dbg_qkvT", [97, NTOK], bf16,
                                  kind="ExternalOutput")
        dbg_abar = nc.dram_tensor("dbg_abar", [3, 128, F * 32], f32,
                                  kind="ExternalOutput")
        dbg_xt = nc.dram_tensor("dbg_xt", [2, 128, NTOK], bf16,
                                kind="ExternalOutput")

    ar_in = nc.dram_tensor("ar_in", [2, 128, NTOK], bf16)
    ar_out = nc.dram_tensor("ar_out", [2, 128, NTOK], bf16, addr_space="Shared")
    ar2_in = nc.dram_tensor("ar2_in", [2, 128, 1], f32)
    ar2_out = nc.dram_tensor("ar2_out", [2, 128, 1], f32, addr_space="Shared")

    inv_sqrt_hd = 1.0 / np.sqrt(np.float32(HD))

    with TileContext(nc) as tc:
        with (
            tc.tile_pool(name="pin", bufs=1) as pin,          # persistent SBUF
            tc.tile_pool(name="work", bufs=3) as work,        # rotating SBUF
            tc.tile_pool(name="estr2", bufs=5) as estr2,
            tc.tile_pool(name="estr4", bufs=7) as estr4,
            tc.tile_pool(name="estr6", bufs=9) as estr6,
            tc.tile_pool(name="ppq", bufs=2, space="PSUM") as ppq,    # 2 banks
            tc.tile_pool(name="pps", bufs=1, space="PSUM") as pps,    # 3 banks
            tc.tile_pool(name="pav", bufs=1, space="PSUM") as pavp,   # 2 banks
            tc.tile_pool(name="ppt", bufs=1, space="PSUM") as ppt,    # 1 bank
        ):
            estr = {2: estr2, 4: estr4, 6: estr6}

            # ---- persistent tiles + weight loads ----
            xt = [pin.tile([128, NTOK], bf16, tag=f"xt{c}", name=f"xt{c}")
                  for c in range(2)]
            wqkv = pin.tile([128, 3 * 2 * 97], bf16, tag="wqkv")
            bqkv = pin.tile([97, 3], f32, tag="bqkv")
            wo3 = pin.tile([128, 256], bf16, tag="wo3")
            abT3 = pin.tile([128, NTOK], bf16, tag="abT3")
            boq = pin.tile([128, 2], f32, tag="boq")
            nhb = pin.tile([1, 1], f32, tag="nhb")
            cinvT2 = pin.tile([128, 3 * NG * 128], f32, tag="cinvT2")
            clsw = pin.tile([128, 2 * NCLS], f32, tag="clsw")
            clsb = pin.tile([1, NCLS], f32, tag="clsb")
            ident = pin.tile([128, 128], f32r, tag="ident")
            ones_row = pin.tile([1, 128], f32, tag="ones_row")
            ones_col = pin.tile([128, 1], f32, tag="ones_col")
            identb = pin.tile([128, 128], bf16, tag="identb")

            for c in range(2):
                nc.sync.dma_start(out=xt[c][:], in_=xt_in[c])
            for si in range(3):
                for c in range(2):
                    nc.sync.dma_start(
                        out=wqkv[:, (si * 2 + c) * 97:(si * 2 + c + 1) * 97],
                        in_=wqkv_in[si, c])
                nc.sync.dma_start(out=bqkv[:, si:si + 1], in_=bqkv_in[si])
                nc.gpsimd.dma_start(
                    out=cinvT2[:, si * NG * 128:(si + 1) * NG * 128],
                    in_=cinvT2_in[si])
            nc.gpsimd.dma_start(out=wo3[:], in_=wo3_in[:])
            for c in range(2):
                nc.gpsimd.dma_start(out=boq[:, c:c + 1], in_=boq_in[c])
                nc.gpsimd.dma_start(out=clsw[:, c * NCLS:(c + 1) * NCLS],
                                    in_=clsw_in[c])
            nc.sync.dma_start(out=nhb[:], in_=nhb_in[:])
            nc.gpsimd.dma_start(out=clsb[:], in_=clsb_in[:])
            nc.gpsimd.dma_start(out=ident[:], in_=id_in[:])
            nc.vector.memset(ones_row[:], 1.0)
            nc.vector.memset(ones_col[:], 1.0)
            nc.vector.tensor_copy(identb[:], ident[:].bitcast(f32))

            qkvT = {s: pin.tile([97, NTOK], bf16, tag=f"qkvT{s}", name=f"qkvT{s}")
                    for s in SCALES}
            q4 = {s: pin.tile([128, NTOK], bf16, tag=f"q4{s}", name=f"q4{s}")
                  for s in SCALES}
            kT4 = {s: pin.tile([128, NTOK], bf16, tag=f"kT4{s}", name=f"kT4{s}")
                   for s in SCALES}
            v4 = {s: pin.tile([128, NTOK], bf16, tag=f"v4{s}", name=f"v4{s}")
                  for s in SCALES}
            vp = {s: pin.tile([128, F * 33], bf16, tag=f"vp{s}", name=f"vp{s}")
                  for s in SCALES}
            abar = {s: pin.tile([128, F * 32], f32, tag=f"abar{s}",
                                name=f"abar{s}") for s in SCALES}
            abT = {s: pin.tile([128, NG * 128], bf16, tag=f"abT{s}",
                               name=f"abT{s}") for s in SCALES}

            # halting state
            ptn = pin.tile([1, F], f32, tag="ptn")
            Rt = pin.tile([1, F], f32, tag="Rt")
            wts = [pin.tile([1, F], f32, tag=f"w{it}", name=f"w{it}")
                   for it in range(2)]
            ssum = [[pin.tile([128, F], f32, tag=f"ssum{it}{c}",
                              name=f"ssum{it}{c}") for c in range(2)]
                    for it in range(2)]
            nc.vector.memset(ptn[:], 0.0)
            nc.vector.memset(Rt[:], 0.0)
            for s in SCALES:
                nc.vector.memset(vp[s][:], 1.0)

            col_cc = _chunks(NTOK)  # [512]*4 + [256]

            def scatter4(dst, src, row0, queue, enable):
                """Group-stack: dst[32r, g*128+q] = src[row0.., (4g+r)*128+q].

                Col block g holds frames 4g..4g+3 stacked on partitions."""
                if not enable:
                    queue.dma_start(out=dst[0:32, :], in_=src[row0:row0 + 32, :])
                    return
                for r in range(4):
                    sv = src[row0:row0 + 32, 0:16 * 128].rearrange(
                        "p (g r q) -> p g r q", r=4, q=128)
                    dv = dst[32 * r:32 * r + 32, 0:512].rearrange(
                        "p (g q) -> p g q", q=128)
                    queue.dma_start(out=dv[:], in_=sv[:, :, r, :])
                for f in (16, 17):
                    r = f % 4
                    queue.dma_start(
                        out=dst[32 * r:32 * r + 32, 512:640],
                        in_=src[row0:row0 + 32, f * 128:(f + 1) * 128])

            for it in range(2):
                # ============ QKV^T projections (all scales) ============
                for si, s in enumerate(SCALES):
                    off = 0
                    for w_cc in col_cc:
                        pq = ppq.tile([128, 512], f32, tag="pq", name="pq")
                        for kc in range(2):
                            nc.tensor.matmul(
                                pq[0:97, :w_cc],
                                wqkv[:, (si * 2 + kc) * 97:(si * 2 + kc + 1) * 97],
                                xt[kc][:, off:off + w_cc],
                                start=(kc == 0), stop=(kc == 1))
                        nc.vector.tensor_scalar_add(
                            out=qkvT[s][:, off:off + w_cc], in0=pq[0:97, :w_cc],
                            scalar1=bqkv[:, si:si + 1])
                        off += w_cc
                    # fan out Q to partition offsets 32/64/96, scatter K/V
                    if R4:
                        for r in range(1, 4):
                            nc.sync.dma_start(out=q4[s][32 * r:32 * r + 32, :],
                                              in_=qkvT[s][0:32, :])
                    scatter4(kT4[s], qkvT[s], 32, nc.gpsimd, R4)
                    scatter4(v4[s], qkvT[s], 64, nc.sync, V4)

                # ============ halting probability ============
                elog = work.tile([1, NTOK], f32, tag="elog", bufs=1)
                nc.scalar.activation(elog[:], qkvT[2][96:97, :],
                                     AF.Exp, bias=nhb[:], scale=-1.0)
                nc.vector.tensor_scalar_add(out=elog[:], in0=elog[:],
                                            scalar1=1.0)
                ptp = ppt.tile([128, F], f32, tag="ptt")
                for f in range(F):
                    nc.tensor.transpose(ptp[:, f:f + 1],
                                        elog[:, f * 128:(f + 1) * 128],
                                        ident[0:1, 0:1].bitcast(f32))
                sig = work.tile([128, F], f32, tag="sig")
                nc.vector.reciprocal(sig[:], ptp[:])
                pp = ppq.tile([128, 512], f32, tag="pq", name="pq")
                nc.tensor.matmul(pp[0:1, :F], ones_col[:],
                                 sig[:], start=True, stop=True)
                p_t = work.tile([1, F], f32, tag="p_t")
                nc.vector.tensor_scalar_mul(out=p_t[:], in0=pp[0:1, :F],
                                            scalar1=1.0 / 128.0)

                run_in = work.tile([1, F], f32, tag="run_in")
                tmp = work.tile([1, F], f32, tag="tmp")
                tmp2 = work.tile([1, F], f32, tag="tmp2")
                nh = work.tile([1, F], f32, tag="nh")
                run = work.tile([1, F], f32, tag="run")
                nc.vector.tensor_scalar(out=run_in[:], in0=ptn[:], scalar1=1.0,
                                        scalar2=None, op0=ALU.is_lt)
                nc.vector.tensor_tensor(out=tmp[:], in0=p_t[:], in1=run_in[:],
                                        op=ALU.mult)
                nc.vector.tensor_tensor(out=tmp2[:], in0=ptn[:], in1=tmp[:],
                                        op=ALU.add)
                nc.vector.tensor_scalar(out=tmp2[:], in0=tmp2[:], scalar1=0.99,
                                        scalar2=None, op0=ALU.is_gt)
                nc.vector.tensor_tensor(out=nh[:], in0=tmp2[:], in1=run_in[:],
                                        op=ALU.mult)
                nc.vector.tensor_tensor(out=run[:], in0=run_in[:], in1=nh[:],
                                        op=ALU.subtract)
                nc.vector.tensor_tensor(out=tmp[:], in0=p_t[:], in1=run[:],
                                        op=ALU.mult)
                nc.vector.tensor_tensor(out=ptn[:], in0=ptn[:], in1=tmp[:],
                                        op=ALU.add)
                nc.vector.tensor_scalar(out=tmp2[:], in0=ptn[:], scalar1=-1.0,
                                        scalar2=1.0, op0=ALU.mult, op1=ALU.add)
                nc.vector.tensor_tensor(out=tmp2[:], in0=nh[:], in1=tmp2[:],
                                        op=ALU.mult)
                nc.vector.tensor_tensor(out=Rt[:], in0=Rt[:], in1=tmp2[:],
                                        op=ALU.add)
                nc.vector.tensor_tensor(out=tmp2[:], in0=nh[:], in1=Rt[:],
                                        op=ALU.mult)
                nc.vector.tensor_tensor(out=ptn[:], in0=ptn[:], in1=tmp2[:],
                                        op=ALU.add)
                nc.vector.tensor_tensor(out=wts[it][:], in0=tmp[:], in1=tmp2[:],
                                        op=ALU.add)

                # ============ attention per scale ============
                for si, s in enumerate(SCALES):
                    nw = F - s + 1
                    sP = s * 128
                    nc.vector.memset(abar[s][:], 0.0)

                    # window pairs (wA, wB) triggered after strip wB+s-1
                    pair_at = {}
                    if C2 == 1:
                        for pi in range(0, nw, 2):
                            wA, wB = pi, (pi + 1 if pi + 1 < nw else None)
                            trig = (wB if wB is not None else wA) + s - 1
                            pair_at[trig] = (wA, wB)
                    else:
                        for pi in range(nw):
                            pair_at[pi + s - 1] = (pi, None)

                    strips = {}
                    for f2 in range(F):
                        # V' via one full 128x128 transpose per 4-frame
                        # group of the group-stacked v4 layout
                        if V4 and f2 % 4 == 0 and f2 // 4 < NG:
                            g = f2 // 4
                            nfr = min(4, F - 4 * g)
                            pv = ppt.tile([128, 128], bf16, tag="ptt",
                                          name="ptt_v")
                            nc.tensor.transpose(
                                pv[:, 0:nfr * 32],
                                v4[s][0:nfr * 32, g * 128:(g + 1) * 128],
                                identb[0:nfr * 32, 0:nfr * 32])
                            pvv = pv[:, 0:nfr * 32].rearrange(
                                "p (f c) -> p f c", c=32)
                            vpv = vp[s][:, 4 * g * 33:(4 * g + nfr) * 33
                                        ].rearrange("p (f c) -> p f c", c=33)
                            nc.vector.tensor_copy(vpv[:, :, 0:32], pvv[:])
                        if (not V4) and f2 % 2 == 0:
                            vfr = [f for f in (f2, f2 + 1) if f < F]
                            pv = ppt.tile([128, 72], bf16, tag="ptt",
                                          name="ptt_v")
                            for fi, f in enumerate(vfr):
                                nc.tensor.transpose(
                                    pv[:, fi * 34:fi * 34 + 32],
                                    v4[s][0:32, f * 128:(f + 1) * 128],
                                    identb[0:32, 0:32])
                            pvv = pv[:, 0:len(vfr) * 34].rearrange(
                                "p (f c) -> p f c", c=34)
                            vpv = vp[s][:, vfr[0] * 33:
                                        (vfr[0] + len(vfr)) * 33].rearrange(
                                "p (f c) -> p f c", c=33)
                            nc.vector.tensor_copy(vpv[:, :, 0:32],
                                                  pvv[:, :, 0:32])

                        # ---- scores strip f2 (4-way row-tiled) ----
                        a = max(0, f2 - s + 1)
                        b = min(F - 1, f2 + s - 1)
                        ncols = (b - a + 1) * 128
                        r = (f2 % 4) if R4 else 0
                        kcol = (f2 // 4) * 128 if R4 else f2 * 128
                        lhs = kT4[s][32 * r:32 * r + 32, kcol:kcol + 128]
                        mov = (qkvT[s] if r == 0 else q4[s])
                        pstr = pps.tile([128, 11 * 128], f32, tag="pstr",
                                        name="pstr")
                        off = 0
                        for w_cc in _chunks(ncols):
                            nc.tensor.matmul(
                                pstr[:, off:off + w_cc], lhs,
                                mov[32 * r:32 * r + 32,
                                    a * 128 + off:a * 128 + off + w_cc],
                                start=True, stop=True,
                                tile_position=(32 * r, 0))
                            off += w_cc
                        est = estr[s].tile([128, (2 * s - 1) * 128], bf16,
                                           tag="est")
                        nc.scalar.activation(est[:, :ncols], pstr[:, :ncols],
                                             AF.Exp, scale=inv_sqrt_hd)
                        strips[f2] = (a, est)

                        if f2 not in pair_at:
                            continue
                        wA, wB = pair_at[f2]
                        if C2 == 2:
                            wA2 = wA % 2
                            wins = [(wA2, wA)]
                        else:
                            wins = [(0, wA)] + ([(1, wB)] if wB is not None else [])
                        # ---- A@V' for the window pair (2-way col-tiled) ----
                        # chunk-outer / window-mid / ji-inner: each (window,
                        # chunk) accumulation group runs uninterrupted in its
                        # bank (start=True clears whole-bank has_written bits),
                        # while the next group overlaps in the other col half
                        # of the PE array.
                        # One sequential accumulation group per (window,
                        # bank); window B's groups run in PE col-groups 2-3 and
                        # overlap window A's later-bank groups.
                        pav = pavp.tile([128, 768], f32, tag="pav", name="pav")
                        av_cc = _chunks(sP)
                        for wi, (w2, w) in enumerate(wins):
                            off = 0
                            for w_cc in av_cc:
                                for ji in range(s):
                                    j = w + ji
                                    aj, ej = strips[j]
                                    qoff = (w - aj) * 128
                                    nc.tensor.matmul(
                                        pav[64 * w2:64 * w2 + 33,
                                            off:off + w_cc],
                                        vp[s][:, j * 33:(j + 1) * 33],
                                        ej[:, qoff + off:qoff + off + w_cc],
                                        start=(ji == 0),
                                        stop=(ji == s - 1),
                                        tile_position=(0, 64 * w2))
                                off += w_cc
                        av_sb = work.tile([128, 768], bf16, tag="av_sb",
                                          bufs=3)
                        use_sc = (wA // 2) % 2 == 0
                        for wi, (w2, w) in enumerate(wins):
                            rb = 64 * w2
                            if use_sc:
                                nc.scalar.copy(av_sb[rb:rb + 33, :sP],
                                               pav[rb:rb + 33, :sP])
                            else:
                                nc.vector.tensor_copy(av_sb[rb:rb + 33, :sP],
                                                      pav[rb:rb + 33, :sP])
                        # ---- transpose to token-major (2-way row-tiled) ----
                        ptw = ppt.tile([128, 408], bf16, tag="ptt",
                                       name="ptt_w")
                        for wi, (w2, w) in enumerate(wins):
                            base = 64 * w2
                            for qc in range(s):
                                nc.tensor.transpose(
                                    ptw[:, (wi * s + qc) * 34:
                                        (wi * s + qc) * 34 + 33],
                                    av_sb[base:base + 33,
                                          qc * 128:(qc + 1) * 128],
                                    identb[base:base + 33, base:base + 33],
                                    tile_position=(base, 0))
                        nwin = len(wins)
                        ptv = ptw[:, 0:nwin * s * 34].rearrange(
                            "p (w c) -> p w c", c=34)
                        rcp = work.tile([128, 12], f32, tag="rcp")
                        nc.vector.reciprocal(rcp[:, :nwin * s],
                                             ptv[:, :, 32])
                        resc = work.tile([128, 12 * 32], f32, tag="resc")
                        rv = resc[:, 0:nwin * s * 32].rearrange(
                            "p (w c) -> p w c", c=32)
                        nc.vector.tensor_tensor(
                            out=rv[:], in0=ptv[:, :, 0:32],
                            in1=rcp[:, :nwin * s].broadcast_to(
                                (128, nwin * s, 32)),
                            op=ALU.mult)
                        for wi, (w2, w) in enumerate(wins):
                            nc.vector.tensor_tensor(
                                out=abar[s][:, w * 32:(w + s) * 32],
                                in0=abar[s][:, w * 32:(w + s) * 32],
                                in1=resc[:, wi * s * 32:(wi + 1) * s * 32],
                                op=ALU.add)

                    # ---- abar -> frame-grouped transpose (4 frames/128) ----
                    for g in range(NG):
                        gw = min(4, F - 4 * g) * 32
                        pg = ppt.tile([128, 128], f32, tag="ptt", name="ptt_g")
                        nc.tensor.transpose(
                            pg[0:gw, :],
                            abar[s][:, g * 128:g * 128 + gw],
                            ident[:].bitcast(f32))
                        nc.vector.tensor_tensor(
                            out=abT[s][0:gw, g * 128:(g + 1) * 128],
                            in0=pg[0:gw, :],
                            in1=cinvT2[0:gw, (si * NG + g) * 128:
                                       (si * NG + g + 1) * 128],
                            op=ALU.mult)
                        for j in range(gw // 32):
                            nc.sync.dma_start(
                                out=abT3[32 * si:32 * si + 32,
                                         (4 * g + j) * 128:
                                         (4 * g + j + 1) * 128],
                                in_=abT[s][32 * j:32 * j + 32,
                                           g * 128:(g + 1) * 128])

                if DBG and it == 0:
                    nc.sync.dma_start(out=dbg_qkvT[:], in_=qkvT[2][:])
                    for si, s in enumerate(SCALES):
                        nc.sync.dma_start(out=dbg_abar[si], in_=abar[s][:])

                # ============ Wo projection (4-way row-tiled) ============
                for hc in range(2):
                    for g in range(NG):
                        nfr = min(4, F - 4 * g)
                        w_cc = nfr * 128
                        pw = ppq.tile([128, 512], f32, tag="pq", name="pq")
                        for j in range(nfr):
                            nc.tensor.matmul(
                                pw[:, j * 128:(j + 1) * 128],
                                wo3[0:96, hc * 128:(hc + 1) * 128],
                                abT3[0:96, (4 * g + j) * 128:
                                     (4 * g + j + 1) * 128],
                                start=True, stop=True)
                        off = g * 512
                        nc.vector.tensor_scalar(
                            out=xt[hc][:, off:off + w_cc], in0=pw[:, :w_cc],
                            scalar1=0.25, scalar2=boq[:, hc:hc + 1],
                            op0=ALU.mult, op1=ALU.add)
                        if it == 0:
                            nc.sync.dma_start(out=ar_in[hc, :, off:off + w_cc],
                                              in_=xt[hc][:, off:off + w_cc])

                if DBG and it == 0:
                    for hc in range(2):
                        nc.sync.dma_start(out=dbg_xt[hc], in_=xt[hc][:])
                if it == 0:
                    nc.gpsimd.collective_compute(
                        "AllReduce", ALU.add,
                        ins=[ar_in[:]], outs=[ar_out[:]],
                        replica_groups=[list(range(NCORES))])
                    for hc in range(2):
                        nc.sync.dma_start(out=xt[hc][:], in_=ar_out[hc])
                for hc in range(2):
                    nc.vector.tensor_reduce(
                        out=ssum[it][hc][:],
                        in_=xt[hc][:].rearrange("p (f q) -> p f q", q=128),
                        axis=mybir.AxisListType.X, op=ALU.add)

            # ============ final combine ============
            w1, w2 = wts[0], wts[1]
            c1 = work.tile([1, 2 * F], f32, tag="coef")
            nc.vector.tensor_copy(c1[:, 0:F], w2[:])
            tmpc = work.tile([1, F], f32, tag="tmpc")
            nc.vector.tensor_scalar(out=tmpc[:], in0=w2[:], scalar1=-1.0,
                                    scalar2=1.0, op0=ALU.mult, op1=ALU.add)
            nc.vector.tensor_tensor(out=tmpc[:], in0=tmpc[:], in1=w1[:],
                                    op=ALU.mult)
            nc.vector.tensor_scalar_mul(out=c1[:, F:2 * F], in0=tmpc[:],
                                        scalar1=1.0 / NCORES)
            pc = ppt.tile([128, 2 * F], f32, tag="ptt", name="ptt_c")
            nc.tensor.matmul(pc[:], ones_row[:], c1[:], start=True, stop=True)
            coefb = work.tile([128, 2 * F], f32, tag="coefb")
            nc.vector.tensor_copy(coefb[:], pc[:])
            vpart = [work.tile([128, 1], f32, tag=f"vpart{hc}",
                               name=f"vpart{hc}") for hc in range(2)]
            for hc in range(2):
                t2 = work.tile([128, F], f32, tag="t2")
                nc.vector.tensor_tensor(out=t2[:], in0=ssum[1][hc][:],
                                        in1=coefb[:, 0:F], op=ALU.mult)
                t1 = work.tile([128, F], f32, tag="t1")
                nc.vector.tensor_tensor(out=t1[:], in0=ssum[0][hc][:],
                                        in1=coefb[:, F:2 * F], op=ALU.mult)
                nc.vector.tensor_tensor(out=t2[:], in0=t2[:], in1=t1[:],
                                        op=ALU.add)
                nc.vector.tensor_reduce(out=vpart[hc][:], in_=t2[:],
                                        axis=mybir.AxisListType.X, op=ALU.add)
                nc.sync.dma_start(out=ar2_in[hc], in_=vpart[hc][:])
            nc.gpsimd.collective_compute(
                "AllReduce", ALU.add,
                ins=[ar2_in[:]], outs=[ar2_out[:]],
                replica_groups=[list(range(NCORES))])
            vfull = [work.tile([128, 1], f32, tag=f"vfull{hc}",
                               name=f"vfull{hc}") for hc in range(2)]
            ob = work.tile([1, NCLS], f32, tag="ob")
            for hc in range(2):
                nc.sync.dma_start(out=vfull[hc][:], in_=ar2_out[hc])
            off = 0
            for w_cc in _chunks(NCLS):
                pcls = ppq.tile([128, 512], f32, tag="pq", name="pq")
                for hc in range(2):
                    nc.tensor.matmul(pcls[0:1, :w_cc], vfull[hc][:],
                                     clsw[:, hc * NCLS + off:
                                          hc * NCLS + off + w_cc],
                                     start=(hc == 0), stop=(hc == 1))
                nc.vector.tensor_tensor(out=ob[:, off:off + w_cc],
                                        in0=pcls[0:1, :w_cc],
                                        in1=clsb[:, off:off + w_cc],
                                        op=ALU.add)
                off += w_cc
            nc.sync.dma_start(out=out_d[:], in_=ob[:])

    nc.compile()
    return nc


_NC_CACHE = None


def _get_nc():
    global _NC_CACHE
    if _NC_CACHE is None:
        _NC_CACHE = build()
    return _NC_CACHE


def _prep_in_maps(inputs):
    emb = np.ascontiguousarray(np.asarray(inputs["multiscale_embed"], np.float32))
    halt_W = np.asarray(inputs["halt_W"], np.float32)
    halt_b = np.asarray(inputs["halt_b"], np.float32)
    cls_W = np.asarray(inputs["cls_W"], np.float32)
    cls_b = np.asarray(inputs["cls_b"], np.float32)
    Wq = np.asarray(inputs["mhsa_Wq"], np.float32)
    bq = np.asarray(inputs["mhsa_bq"], np.float32)
    Wk = np.asarray(inputs["mhsa_Wk"], np.float32)
    bk = np.asarray(inputs["mhsa_bk"], np.float32)
    Wv = np.asarray(inputs["mhsa_Wv"], np.float32)
    bv = np.asarray(inputs["mhsa_bv"], np.float32)
    Wo = np.asarray(inputs["mhsa_Wo"], np.float32)
    bo = np.asarray(inputs["mhsa_bo"], np.float32)

    import ml_dtypes
    bf = ml_dtypes.bfloat16
    xt = np.ascontiguousarray(
        emb.reshape(NTOK, H).T.reshape(2, 128, NTOK)).astype(bf)
    boq = np.ascontiguousarray(
        (0.25 * bo.sum(axis=0)).reshape(2, 128, 1))
    hwc = halt_W.reshape(2, 128)
    nhb = np.full((1, 1), -float(halt_b[0]), np.float32)
    # cinvT2[si]: partition (j*32+d), col g*128+t -> 1/counts_s[4g+j]
    cinvT2 = np.zeros((3, 128, NG * 128), np.float32)
    for si, s in enumerate(SCALES):
        cs = 1.0 / _counts(s)
        for g in range(NG):
            for j in range(min(4, F - 4 * g)):
                cinvT2[si, j * 32:(j + 1) * 32,
                       g * 128:(g + 1) * 128] = cs[4 * g + j]
    clsw = np.ascontiguousarray(cls_W.reshape(2, 128, NCLS))
    clsb = cls_b.reshape(1, NCLS).astype(np.float32)
    ident = np.eye(128, dtype=np.float32)

    in_maps = []
    for h in range(NCORES):
        sl = slice(h * HD, (h + 1) * HD)
        wqkv = np.zeros((3, 2, 128, 97), bf)
        bqkv = np.zeros((3, 97, 1), np.float32)
        wo3 = np.zeros((128, 256), bf)
        for si in range(3):
            blk = np.concatenate(
                [Wq[si][:, sl], Wk[si][:, sl], Wv[si][:, sl]], axis=1)
            wqkv[si, :, :, :96] = blk.reshape(2, 128, 96).astype(bf)
            if si == 0:
                wqkv[si, :, :, 96] = hwc.astype(bf)
            bqkv[si, :96] = np.concatenate(
                [bq[si][sl], bk[si][sl], bv[si][sl]])[:, None]
            for hc in range(2):
                wo3[32 * si:32 * si + 32, hc * 128:(hc + 1) * 128] = \
                    Wo[si][sl, hc * 128:(hc + 1) * 128].astype(bf)
        in_maps.append({
            "xt": xt, "wqkv": wqkv, "bqkv": bqkv, "wo3": wo3, "boq": boq,
            "nhb": nhb, "cinvT2": cinvT2, "clsw": clsw, "clsb": clsb,
            "ident": ident,
        })
    return in_maps


def run(inputs, trace=False):
    _install_ntff_hook()
    from concourse.bass_utils import run_bass_kernel_spmd

    nc = _get_nc()
    in_maps = _prep_in_maps(inputs)
    res = run_bass_kernel_spmd(nc, in_maps, list(range(NCORES)), trace=trace)
    out = np.asarray(res.results[0]["out"], np.float32)
    return out, res


def kernel(**inputs):
    out, _ = run(inputs, trace=False)
    return out
